# revision 13
# baseline (speedup 1.0000x reference)
"""Multi-head self-attention (B=4, S=2048, D=768, H=12, dh=64) on 8 trn2 cores.

Sharding: core = b*2 + g  (b = batch 0..3, g = head-group of 6 heads).
Each core computes q/k/v projections for its 6 heads over the full sequence,
masked softmax attention, and a partial output projection (column slice of
o_w => row-parallel). Host sums the two partial outputs per batch element.

Key points:
  - mask gather: only unmasked k positions (padded to a multiple of 128) are
    shipped/projected/exp'd; padding columns get a -1e30 per-partition bias
    inside the ACT exp instruction (out = exp(scale*s + bias)).
  - scoresT [k, q] layout so softmax weights feed the context matmul as lhsT
    with no transpose; softmax denominators come free from an appended
    ones-column in v (psum row 64 of the context matmul).
  - per-head normalization deferred: unnormalized ctx + sums copied to SBUF,
    one reciprocal op, PE rank-1 broadcast of recip across partitions, one
    tensor_tensor multiply per head.
  - biases: q/k bias = per-partition DVE tensor_scalar on psum eviction;
    v bias via contraction-augmentation (ones row in xvT, v_b row in wvT);
    o_b broadcast across partitions once via PE rank-1, added on psum evict
    (zeros passed for the g==1 cores so the host sum applies it once).
"""

import numpy as np
import ml_dtypes

import concourse.bass as bass
import concourse.mybir as mybir
import concourse.tile as tile
from concourse import bacc
from concourse.bass_utils import run_bass_kernel_spmd

BS, SEQ, DIM, NH = 4, 2048, 768, 12
DH = 64
HEADS = 6            # heads per core
DGRP = HEADS * DH    # 384
N_CORES = 8
P = 128

F32 = mybir.dt.float32
BF16 = mybir.dt.bfloat16

# matmul dtype config (bf16 = 4x faster PE than fp32)
MM_DT = BF16
MM_NP = ml_dtypes.bfloat16 if MM_DT == BF16 else np.float32

NEG = -1.0e30


def _build(NKV: int):
    """Build the per-core Bass program, parameterized by padded kv length."""
    KC = NKV // P          # k chunks
    QC = SEQ // P          # 16
    NT = SEQ // 512        # 4 q-slices
    KIN = DIM // P         # 6 contraction chunks for q/k proj
    KIN_V = 7              # 768 inputs + ones row, padded to 896

    nc = bacc.Bacc(None, target_bir_lowering=False, debug=False)

    xqT = nc.declare_dram_parameter("xqT", [DIM, SEQ], MM_DT, isOutput=False)
    xkT = nc.declare_dram_parameter("xkT", [DIM, NKV], MM_DT, isOutput=False)
    xvT = nc.declare_dram_parameter("xvT", [P * KIN_V, NKV], MM_DT, isOutput=False)
    wqT = nc.declare_dram_parameter("wqT", [DIM, DGRP], MM_DT, isOutput=False)
    wkT = nc.declare_dram_parameter("wkT", [DIM, DGRP], MM_DT, isOutput=False)
    wvT = nc.declare_dram_parameter("wvT", [P * KIN_V, DGRP], MM_DT, isOutput=False)
    woT = nc.declare_dram_parameter("woT", [DGRP, DIM], MM_DT, isOutput=False)
    qb = nc.declare_dram_parameter("qb", [DGRP], F32, isOutput=False)
    kb = nc.declare_dram_parameter("kb", [DGRP], F32, isOutput=False)
    ob = nc.declare_dram_parameter("ob", [DIM], F32, isOutput=False)
    pb = nc.declare_dram_parameter("pb", [NKV], F32, isOutput=False)
    out = nc.declare_dram_parameter("out", [SEQ, DIM], F32, isOutput=True)

    xqT_r = xqT.rearrange("(kk pi) n -> pi kk n", pi=P)
    xkT_r = xkT.rearrange("(kk pi) n -> pi kk n", pi=P)
    xvT_r = xvT.rearrange("(kk pi) n -> pi kk n", pi=P)
    wqT_r = wqT.rearrange("(kk pi) n -> pi kk n", pi=P)
    wkT_r = wkT.rearrange("(kk pi) n -> pi kk n", pi=P)
    wvT_r = wvT.rearrange("(kk pi) n -> pi kk n", pi=P)
    woT_r = woT.rearrange("(kk pi) n -> pi kk n", pi=P)
    qb_r = qb.rearrange("(m pi) -> pi m", pi=P)
    kb_r = kb.rearrange("(m pi) -> pi m", pi=P)
    pb_r = pb.rearrange("(c pi) -> pi c", pi=P)

    with tile.TileContext(nc) as tc:
        with (
            tc.tile_pool(name="const", bufs=1) as const,
            tc.tile_pool(name="persist", bufs=1) as persist,
            tc.tile_pool(name="expp", bufs=2) as expp,
            tc.tile_pool(name="outp", bufs=2) as outp,
        ):
            # ---- constants ----
            pb_sb = const.tile([P, KC], F32)
            nc.sync.dma_start(pb_sb[:], pb_r)
            qb_sb = const.tile([P, 3], F32)
            nc.sync.dma_start(qb_sb[:], qb_r)
            kb_sb = const.tile([P, 3], F32)
            nc.sync.dma_start(kb_sb[:], kb_r)
            wo_sb = const.tile([P, 3, DIM], MM_DT)
            nc.sync.dma_start(wo_sb[:], woT_r)
            ones_sb = const.tile([1, P], F32)
            nc.vector.memset(ones_sb[:], 1.0)
            ob_row = const.tile([1, DIM], F32)
            nc.sync.dma_start(ob_row[:], ob[None, :])
            ob_bc = const.tile([P, DIM], F32)

            # ---- persistent activations ----
            qT_sb = persist.tile([P, 3, SEQ], MM_DT)
            kT_sb = persist.tile([P, 3, NKV], MM_DT)
            v_sb = persist.tile([P, KC, HEADS * 65], MM_DT)
            ctxu_sb = persist.tile([P, 3, SEQ], MM_DT)
            ctx_sb = persist.tile([P, 3, SEQ], MM_DT)

            # ones column per head in v (gives softmax sums in psum row 64)
            for h in range(HEADS):
                nc.vector.memset(v_sb[:, :, 65 * h + 64], 1.0)

            with tc.tile_pool(name="wpool", bufs=1) as wpool, \
                 tc.tile_pool(name="xslice", bufs=2) as xslice, \
                 tc.tile_pool(name="psA", bufs=3, space="PSUM") as psA:
                # o_b broadcast across partitions (rank-1 matmul)
                for n0, nsz in ((0, 512), (512, 256)):
                    ps = psA.tile([P, 512], F32, tag="psA0")
                    nc.tensor.matmul(ps[:, 0:nsz], ones_sb[:],
                                     ob_row[:, n0:n0 + nsz],
                                     start=True, stop=True)
                    nc.vector.tensor_copy(out=ob_bc[:, n0:n0 + nsz],
                                          in_=ps[:, 0:nsz])

                wq_sb = wpool.tile([P, KIN, DGRP], MM_DT)
                nc.sync.dma_start(wq_sb[:], wqT_r)
                wk_sb = wpool.tile([P, KIN, DGRP], MM_DT)
                nc.sync.dma_start(wk_sb[:], wkT_r)
                wv_sb = wpool.tile([P, KIN_V, DGRP], MM_DT)
                nc.sync.dma_start(wv_sb[:], wvT_r)

                # ---- q projection: qT[384, 2048] = wqT.T @ xqT (+qb) ----
                for nt in range(NT):
                    xq_t = xslice.tile([P, KIN, 512], MM_DT, tag="xq")
                    nc.sync.dma_start(xq_t[:], xqT_r[:, :, nt * 512:(nt + 1) * 512])
                    for m in range(3):
                        ps = psA.tile([P, 512], F32, tag=f"psA{(nt * 3 + m) % 2}")
                        for kk in range(KIN):
                            nc.tensor.matmul(
                                ps[:],
                                wq_sb[:, kk, m * P:(m + 1) * P],
                                xq_t[:, kk, :],
                                start=(kk == 0), stop=(kk == KIN - 1),
                            )
                        nc.vector.tensor_scalar_add(
                            qT_sb[:, m, nt * 512:(nt + 1) * 512], ps[:],
                            qb_sb[:, m, None],
                        )

                # ---- k projection: kT[384, NKV] ----
                KSL = 384
                for nt in range(NKV // KSL):
                    xk_t = xslice.tile([P, KIN, KSL], MM_DT, tag="xk")
                    nc.sync.dma_start(xk_t[:], xkT_r[:, :, nt * KSL:(nt + 1) * KSL])
                    for m in range(3):
                        ps = psA.tile([P, KSL], F32, tag=f"psA{(nt * 3 + m) % 2}")
                        for kk in range(KIN):
                            nc.tensor.matmul(
                                ps[:],
                                wk_sb[:, kk, m * P:(m + 1) * P],
                                xk_t[:, kk, :],
                                start=(kk == 0), stop=(kk == KIN - 1),
                            )
                        nc.vector.tensor_scalar_add(
                            kT_sb[:, m, nt * KSL:(nt + 1) * KSL], ps[:],
                            kb_sb[:, m, None],
                        )

                # ---- v projection: v[NKV, 384] (k rows on partitions) ----
                for m in range(KC):
                    xv_t = xslice.tile([P, KIN_V, P], MM_DT, tag="xv")
                    nc.sync.dma_start(xv_t[:], xvT_r[:, :, m * P:(m + 1) * P])
                    ps = psA.tile([P, DGRP], F32, tag=f"psA{m % 2}")
                    for kk in range(KIN_V):
                        nc.tensor.matmul(
                            ps[:],
                            xv_t[:, kk, :],
                            wv_sb[:, kk, :],
                            start=(kk == 0), stop=(kk == KIN_V - 1),
                        )
                    for h in range(HEADS):
                        nc.vector.tensor_copy(
                            out=v_sb[:, m, 65 * h:65 * h + 64],
                            in_=ps[:, 64 * h:64 * h + 64],
                        )

            # ---- attention per head ----
            with tc.tile_pool(name="psS", bufs=1, space="PSUM") as psS, \
                 tc.tile_pool(name="psC", bufs=1, space="PSUM") as psC, \
                 tc.tile_pool(name="stat", bufs=2) as stat:
                for h in range(HEADS):
                    chunk, off = h // 2, 64 * (h % 2)
                    ps_ctx = psC.tile([P, SEQ], F32, tag="ctx")
                    for kc in range(KC):
                        ps_s = psS.tile([P, SEQ], F32, tag="s")
                        for qt in range(NT):
                            nc.tensor.matmul(
                                ps_s[:, qt * 512:(qt + 1) * 512],
                                kT_sb[off:off + DH, chunk, kc * P:(kc + 1) * P],
                                qT_sb[off:off + DH, chunk, qt * 512:(qt + 1) * 512],
                                start=True, stop=True,
                            )
                        exp_t = expp.tile([P, SEQ], MM_DT, tag="exp")
                        nc.scalar.activation(
                            exp_t[:], ps_s[:], mybir.ActivationFunctionType.Exp,
                            bias=pb_sb[:, kc, None], scale=0.125,
                        )
                        for qt in range(NT):
                            nc.tensor.matmul(
                                ps_ctx[0:65, qt * 512:(qt + 1) * 512],
                                v_sb[:, kc, 65 * h:65 * h + 65],
                                exp_t[:, qt * 512:(qt + 1) * 512],
                                start=(kc == 0), stop=(kc == KC - 1),
                            )
                    # evict unnormalized ctx + sums, then normalize: broadcast
                    # 1/sums into rows 64..127 of the ctx psum tile (free once
                    # the sums row is copied out) via a rank-1 matmul
                    nc.vector.tensor_copy(
                        out=ctxu_sb[off:off + DH, chunk, :], in_=ps_ctx[0:64, :],
                    )
                    sums_t = stat.tile([1, SEQ], F32, tag="sums")
                    nc.vector.tensor_copy(out=sums_t[:], in_=ps_ctx[64:65, :])
                    recip_t = stat.tile([1, SEQ], F32, tag="recip")
                    nc.vector.reciprocal(recip_t[:], sums_t[:])
                    for qt in range(NT):
                        nc.tensor.matmul(
                            ps_ctx[64:128, qt * 512:(qt + 1) * 512],
                            ones_sb[:, 0:DH],
                            recip_t[:, qt * 512:(qt + 1) * 512],
                            start=True, stop=True,
                        )
                    nc.vector.tensor_tensor(
                        ctx_sb[off:off + DH, chunk, :],
                        ctxu_sb[off:off + DH, chunk, :],
                        ps_ctx[64:128, :],
                        mybir.AluOpType.mult,
                    )

            # ---- output projection: out[2048, 768] (+ o_b) ----
            with tc.tile_pool(name="psO", bufs=3, space="PSUM") as psO:
                for qc in range(QC):
                    o_t = outp.tile([P, DIM], F32, tag="o")
                    for ntile, n0, nsz in ((0, 0, 512), (1, 512, 256)):
                        ps = psO.tile([P, 512], F32, tag=f"psO{(qc * 2 + ntile) % 2}")
                        for kk in range(3):
                            nc.tensor.matmul(
                                ps[:, 0:nsz],
                                ctx_sb[:, kk, qc * P:(qc + 1) * P],
                                wo_sb[:, kk, n0:n0 + nsz],
                                start=(kk == 0), stop=(kk == 2),
                            )
                        nc.vector.tensor_tensor(
                            o_t[:, n0:n0 + nsz], ps[:, 0:nsz], ob_bc[:, n0:n0 + nsz],
                            mybir.AluOpType.add,
                        )
                    nc.sync.dma_start(out[qc * P:(qc + 1) * P, :], o_t[:])

    nc.compile()
    return nc


_cache: dict = {}

# test harnesses may set e.g. {"trace": True, "tmpdir": ...}; empty for grading
_run_opts: dict = {}
LAST_RES = None


def _get_nc(NKV: int):
    if NKV not in _cache:
        _cache[NKV] = _build(NKV)
    return _cache[NKV]


def kernel(query, key_, value, mask, q_w, q_b, k_w, k_b, v_w, v_b, o_w, o_b):
    query = np.asarray(query, np.float32)
    key_ = np.asarray(key_, np.float32)
    value = np.asarray(value, np.float32)
    mask = np.asarray(mask)
    q_w = np.asarray(q_w, np.float32)
    q_b = np.asarray(q_b, np.float32)
    k_w = np.asarray(k_w, np.float32)
    k_b = np.asarray(k_b, np.float32)
    v_w = np.asarray(v_w, np.float32)
    v_b = np.asarray(v_b, np.float32)
    o_w = np.asarray(o_w, np.float32)
    o_b = np.asarray(o_b, np.float32)

    counts = (mask != 0).sum(axis=1)
    NKV = max(P, int(-(-int(counts.max()) // P) * P))
    nc = _get_nc(NKV)

    zeros_ob = np.zeros_like(o_b)
    in_maps = []
    for b in range(BS):
        idx = np.nonzero(mask[b])[0]
        cnt = len(idx)
        xk_g = np.zeros((NKV, DIM), np.float32)
        xv_g = np.zeros((NKV, DIM), np.float32)
        xk_g[:cnt] = key_[b][idx]
        xv_g[:cnt] = value[b][idx]
        xqT_b = np.ascontiguousarray(query[b].T).astype(MM_NP)
        xkT_b = np.ascontiguousarray(xk_g.T).astype(MM_NP)
        xvT_b = np.zeros((P * 7, NKV), MM_NP)
        xvT_b[:DIM] = xv_g.T
        xvT_b[DIM] = 1.0
        pb_b = np.where(np.arange(NKV) < cnt, 0.0, NEG).astype(np.float32)
        for g in range(2):
            sl = slice(DGRP * g, DGRP * (g + 1))
            in_maps.append({
                "xqT": xqT_b,
                "xkT": xkT_b,
                "xvT": xvT_b,
                "wqT": np.ascontiguousarray(q_w[sl].T).astype(MM_NP),
                "wkT": np.ascontiguousarray(k_w[sl].T).astype(MM_NP),
                "wvT": np.concatenate(
                    [v_w[sl].T, v_b[None, sl],
                     np.zeros((P - 1, DGRP), np.float32)], axis=0).astype(MM_NP),
                "woT": np.ascontiguousarray(o_w[:, sl].T).astype(MM_NP),
                "qb": q_b[sl].copy(),
                "kb": k_b[sl].copy(),
                "ob": o_b if g == 0 else zeros_ob,
                "pb": pb_b,
            })

    res = run_bass_kernel_spmd(nc, in_maps, core_ids=list(range(N_CORES)),
                               **_run_opts)
    global LAST_RES
    LAST_RES = res
    out = np.empty((BS, SEQ, DIM), np.float32)
    for b in range(BS):
        out[b] = res.results[2 * b]["out"] + res.results[2 * b + 1]["out"]
    return out


# revision 18
# speedup vs baseline: 1.0906x; 1.0906x over previous
"""Multi-head self-attention (B=4, S=2048, D=768, H=12, dh=64) on 8 trn2 cores.

Sharding: core = b*2 + g  (b = batch 0..3, g = head-group of 6 heads).
Each core computes q/k/v projections for its 6 heads over the full sequence,
masked softmax attention, and a partial output projection (column slice of
o_w => row-parallel). Host sums the two partial outputs per batch element.

Key points:
  - mask gather: only unmasked k positions (padded to a multiple of 128) are
    shipped/projected/exp'd; padding columns get a -1e30 per-partition bias
    inside the ACT exp instruction (out = exp(scale*s + bias)).
  - scoresT [k, q] layout so softmax weights feed the context matmul as lhsT
    with no transpose; softmax denominators come free from an appended
    ones-column in v (psum row 64 of the context matmul).
  - per-head normalization deferred: unnormalized ctx + sums copied to SBUF,
    one reciprocal op, PE rank-1 broadcast of recip across partitions, one
    tensor_tensor multiply per head.
  - biases: q/k bias = per-partition DVE tensor_scalar on psum eviction;
    v bias via contraction-augmentation (ones row in xvT, v_b row in wvT);
    o_b broadcast across partitions once via PE rank-1, added on psum evict
    (zeros passed for the g==1 cores so the host sum applies it once).
"""

import numpy as np
import ml_dtypes

import concourse.bass as bass
import concourse.mybir as mybir
import concourse.tile as tile
from concourse import bacc
from concourse.bass_utils import run_bass_kernel_spmd

BS, SEQ, DIM, NH = 4, 2048, 768, 12
DH = 64
HEADS = 6            # heads per core
DGRP = HEADS * DH    # 384
N_CORES = 8
P = 128

F32 = mybir.dt.float32
F32R = mybir.dt.float32r  # fp32 I/O, ~1.5e-4 matmul precision, 4x faster than fp32
BF16 = mybir.dt.bfloat16

# matmul dtype config (bf16 = 4x faster PE than fp32)
MM_DT = BF16
MM_NP = ml_dtypes.bfloat16 if MM_DT == BF16 else np.float32

NEG = -1.0e30


def _build(NKV: int):
    """Build the per-core Bass program, parameterized by padded kv length."""
    KC = NKV // P          # k chunks
    QC = SEQ // P          # 16
    NT = SEQ // 512        # 4 q-slices
    KIN = DIM // P         # 6 contraction chunks for q/k proj
    KIN_V = 7              # 768 inputs + ones row, padded to 896

    nc = bacc.Bacc(None, target_bir_lowering=False, debug=False)

    xqT = nc.declare_dram_parameter("xqT", [DIM, SEQ], MM_DT, isOutput=False)
    xkT = nc.declare_dram_parameter("xkT", [DIM, NKV], MM_DT, isOutput=False)
    xvT = nc.declare_dram_parameter("xvT", [P * KIN_V, NKV], MM_DT, isOutput=False)
    wqT = nc.declare_dram_parameter("wqT", [DIM, DGRP], MM_DT, isOutput=False)
    wkT = nc.declare_dram_parameter("wkT", [DIM, DGRP], MM_DT, isOutput=False)
    wvT = nc.declare_dram_parameter("wvT", [P * KIN_V, DGRP], MM_DT, isOutput=False)
    woT = nc.declare_dram_parameter("woT", [DGRP, DIM], MM_DT, isOutput=False)
    qb = nc.declare_dram_parameter("qb", [DGRP], F32, isOutput=False)
    kb = nc.declare_dram_parameter("kb", [DGRP], F32, isOutput=False)
    ob = nc.declare_dram_parameter("ob", [DIM], F32, isOutput=False)
    pb = nc.declare_dram_parameter("pb", [NKV], F32, isOutput=False)
    out = nc.declare_dram_parameter("out", [SEQ, DIM], F32, isOutput=True)

    xqT_r = xqT.rearrange("(kk pi) n -> pi kk n", pi=P)
    xkT_r = xkT.rearrange("(kk pi) n -> pi kk n", pi=P)
    xvT_r = xvT.rearrange("(kk pi) n -> pi kk n", pi=P)
    wqT_r = wqT.rearrange("(kk pi) n -> pi kk n", pi=P)
    wkT_r = wkT.rearrange("(kk pi) n -> pi kk n", pi=P)
    wvT_r = wvT.rearrange("(kk pi) n -> pi kk n", pi=P)
    woT_r = woT.rearrange("(kk pi) n -> pi kk n", pi=P)
    qb_r = qb.rearrange("(m pi) -> pi m", pi=P)
    kb_r = kb.rearrange("(m pi) -> pi m", pi=P)
    pb_r = pb.rearrange("(c pi) -> pi c", pi=P)

    with tile.TileContext(nc) as tc:
        with (
            tc.tile_pool(name="const", bufs=1) as const,
            tc.tile_pool(name="persist", bufs=1) as persist,
            tc.tile_pool(name="expp", bufs=2) as expp,
            tc.tile_pool(name="outp", bufs=2) as outp,
        ):
            # ---- constants ----
            pb_sb = const.tile([P, KC], F32)
            nc.sync.dma_start(pb_sb[:], pb_r)
            qb_sb = const.tile([P, 3], F32)
            nc.sync.dma_start(qb_sb[:], qb_r)
            kb_sb = const.tile([P, 3], F32)
            nc.sync.dma_start(kb_sb[:], kb_r)
            wo_sb = const.tile([P, 3, DIM], MM_DT)
            nc.sync.dma_start(wo_sb[:], woT_r)
            ones_sb = const.tile([1, P], F32)
            nc.vector.memset(ones_sb[:], 1.0)
            ob_row = const.tile([1, DIM], F32)
            nc.sync.dma_start(ob_row[:], ob[None, :])
            ob_bc = const.tile([P, DIM], F32)

            # ---- persistent activations ----
            qT_sb = persist.tile([P, 3, SEQ], MM_DT)
            kT_sb = persist.tile([P, 3, NKV], MM_DT)
            v_sb = persist.tile([P, KC, HEADS * 65], MM_DT)
            ctxu_sb = persist.tile([P, 3, SEQ], MM_DT)
            ctx_sb = persist.tile([P, 3, SEQ], MM_DT)

            # ones column per head in v (gives softmax sums in psum row 64)
            for h in range(HEADS):
                nc.vector.memset(v_sb[:, :, 65 * h + 64], 1.0)

            with tc.tile_pool(name="wpool", bufs=1) as wpool, \
                 tc.tile_pool(name="xslice", bufs=2) as xslice, \
                 tc.tile_pool(name="psA", bufs=3, space="PSUM") as psA:
                # o_b broadcast across partitions (rank-1 matmul)
                for n0, nsz in ((0, 512), (512, 256)):
                    ps = psA.tile([P, 512], F32, tag="psA0")
                    nc.tensor.matmul(ps[:, 0:nsz], ones_sb[:],
                                     ob_row[:, n0:n0 + nsz],
                                     start=True, stop=True)
                    nc.vector.tensor_copy(out=ob_bc[:, n0:n0 + nsz],
                                          in_=ps[:, 0:nsz])

                wq_sb = wpool.tile([P, KIN, DGRP], MM_DT)
                nc.sync.dma_start(wq_sb[:], wqT_r)
                wk_sb = wpool.tile([P, KIN, DGRP], MM_DT)
                nc.sync.dma_start(wk_sb[:], wkT_r)
                wv_sb = wpool.tile([P, KIN_V, DGRP], MM_DT)
                nc.sync.dma_start(wv_sb[:], wvT_r)

                # ---- q projection: qT[384, 2048] = wqT.T @ xqT (+qb) ----
                for nt in range(NT):
                    xq_t = xslice.tile([P, KIN, 512], MM_DT, tag="xq")
                    nc.sync.dma_start(xq_t[:], xqT_r[:, :, nt * 512:(nt + 1) * 512])
                    for m in range(3):
                        ps = psA.tile([P, 512], F32, tag=f"psA{(nt * 3 + m) % 2}")
                        for kk in range(KIN):
                            nc.tensor.matmul(
                                ps[:],
                                wq_sb[:, kk, m * P:(m + 1) * P],
                                xq_t[:, kk, :],
                                start=(kk == 0), stop=(kk == KIN - 1),
                            )
                        nc.vector.tensor_scalar_add(
                            qT_sb[:, m, nt * 512:(nt + 1) * 512], ps[:],
                            qb_sb[:, m, None],
                        )

                # ---- k projection: kT[384, NKV] ----
                KSL = 384
                for nt in range(NKV // KSL):
                    xk_t = xslice.tile([P, KIN, KSL], MM_DT, tag="xk")
                    nc.sync.dma_start(xk_t[:], xkT_r[:, :, nt * KSL:(nt + 1) * KSL])
                    for m in range(3):
                        ps = psA.tile([P, KSL], F32, tag=f"psA{(nt * 3 + m) % 2}")
                        for kk in range(KIN):
                            nc.tensor.matmul(
                                ps[:],
                                wk_sb[:, kk, m * P:(m + 1) * P],
                                xk_t[:, kk, :],
                                start=(kk == 0), stop=(kk == KIN - 1),
                            )
                        nc.vector.tensor_scalar_add(
                            kT_sb[:, m, nt * KSL:(nt + 1) * KSL], ps[:],
                            kb_sb[:, m, None],
                        )

                # ---- v projection: v[NKV, 384] (k rows on partitions) ----
                for m in range(KC):
                    xv_t = xslice.tile([P, KIN_V, P], MM_DT, tag="xv")
                    nc.sync.dma_start(xv_t[:], xvT_r[:, :, m * P:(m + 1) * P])
                    ps = psA.tile([P, DGRP], F32, tag=f"psA{m % 2}")
                    for kk in range(KIN_V):
                        nc.tensor.matmul(
                            ps[:],
                            xv_t[:, kk, :],
                            wv_sb[:, kk, :],
                            start=(kk == 0), stop=(kk == KIN_V - 1),
                        )
                    for h in range(HEADS):
                        nc.vector.tensor_copy(
                            out=v_sb[:, m, 65 * h:65 * h + 64],
                            in_=ps[:, 64 * h:64 * h + 64],
                        )

            # ---- attention per head ----
            with tc.tile_pool(name="psS", bufs=1, space="PSUM") as psS, \
                 tc.tile_pool(name="psC", bufs=1, space="PSUM") as psC, \
                 tc.tile_pool(name="stat", bufs=2) as stat:
                for h in range(HEADS):
                    chunk, off = h // 2, 64 * (h % 2)
                    ps_ctx = psC.tile([P, SEQ], F32, tag="ctx")
                    for kc in range(KC):
                        ps_s = psS.tile([P, SEQ], F32, tag="s")
                        for qt in range(NT):
                            nc.tensor.matmul(
                                ps_s[:, qt * 512:(qt + 1) * 512],
                                kT_sb[off:off + DH, chunk, kc * P:(kc + 1) * P],
                                qT_sb[off:off + DH, chunk, qt * 512:(qt + 1) * 512],
                                start=True, stop=True,
                            )
                        exp_t = expp.tile([P, SEQ], MM_DT, tag="exp")
                        nc.scalar.activation(
                            exp_t[:], ps_s[:], mybir.ActivationFunctionType.Exp,
                            bias=pb_sb[:, kc, None], scale=0.125,
                        )
                        for qt in range(NT):
                            nc.tensor.matmul(
                                ps_ctx[0:65, qt * 512:(qt + 1) * 512],
                                v_sb[:, kc, 65 * h:65 * h + 65],
                                exp_t[:, qt * 512:(qt + 1) * 512],
                                start=(kc == 0), stop=(kc == KC - 1),
                            )
                    # evict unnormalized ctx + sums, then normalize: broadcast
                    # 1/sums into rows 64..127 of the ctx psum tile (free once
                    # the sums row is copied out) via a rank-1 matmul
                    nc.vector.tensor_copy(
                        out=ctxu_sb[off:off + DH, chunk, :], in_=ps_ctx[0:64, :],
                    )
                    sums_t = stat.tile([1, SEQ], F32, tag="sums")
                    nc.vector.tensor_copy(out=sums_t[:], in_=ps_ctx[64:65, :])
                    recip_t = stat.tile([1, SEQ], F32, tag="recip")
                    nc.vector.reciprocal_approx_fast(out=recip_t[:], in_=sums_t[:])
                    for qt in range(NT):
                        nc.tensor.matmul(
                            ps_ctx[64:128, qt * 512:(qt + 1) * 512],
                            ones_sb[:, 0:DH],
                            recip_t[:, qt * 512:(qt + 1) * 512],
                            start=True, stop=True,
                        )
                    nc.vector.tensor_tensor(
                        ctx_sb[off:off + DH, chunk, :],
                        ctxu_sb[off:off + DH, chunk, :],
                        ps_ctx[64:128, :],
                        mybir.AluOpType.mult,
                    )

            # ---- output projection: out[2048, 768] (+ o_b) ----
            with tc.tile_pool(name="psO", bufs=3, space="PSUM") as psO:
                for qc in range(QC):
                    o_t = outp.tile([P, DIM], F32, tag="o")
                    for ntile, n0, nsz in ((0, 0, 512), (1, 512, 256)):
                        ps = psO.tile([P, 512], F32, tag=f"psO{(qc * 2 + ntile) % 2}")
                        for kk in range(3):
                            nc.tensor.matmul(
                                ps[:, 0:nsz],
                                ctx_sb[:, kk, qc * P:(qc + 1) * P],
                                wo_sb[:, kk, n0:n0 + nsz],
                                start=(kk == 0), stop=(kk == 2),
                            )
                        nc.vector.tensor_tensor(
                            o_t[:, n0:n0 + nsz], ps[:, 0:nsz], ob_bc[:, n0:n0 + nsz],
                            mybir.AluOpType.add,
                        )
                    nc.sync.dma_start(out[qc * P:(qc + 1) * P, :], o_t[:])

    nc.compile()
    return nc


_cache: dict = {}

# test harnesses may set e.g. {"trace": True, "tmpdir": ...}; empty for grading
_run_opts: dict = {}
LAST_RES = None


def _get_nc(NKV: int):
    if NKV not in _cache:
        _cache[NKV] = _build(NKV)
    return _cache[NKV]


def kernel(query, key_, value, mask, q_w, q_b, k_w, k_b, v_w, v_b, o_w, o_b):
    query = np.asarray(query, np.float32)
    key_ = np.asarray(key_, np.float32)
    value = np.asarray(value, np.float32)
    mask = np.asarray(mask)
    q_w = np.asarray(q_w, np.float32)
    q_b = np.asarray(q_b, np.float32)
    k_w = np.asarray(k_w, np.float32)
    k_b = np.asarray(k_b, np.float32)
    v_w = np.asarray(v_w, np.float32)
    v_b = np.asarray(v_b, np.float32)
    o_w = np.asarray(o_w, np.float32)
    o_b = np.asarray(o_b, np.float32)

    counts = (mask != 0).sum(axis=1)
    NKV = max(P, int(-(-int(counts.max()) // P) * P))
    nc = _get_nc(NKV)

    zeros_ob = np.zeros_like(o_b)
    in_maps = []
    for b in range(BS):
        idx = np.nonzero(mask[b])[0]
        cnt = len(idx)
        xk_g = np.zeros((NKV, DIM), np.float32)
        xv_g = np.zeros((NKV, DIM), np.float32)
        xk_g[:cnt] = key_[b][idx]
        xv_g[:cnt] = value[b][idx]
        xqT_b = np.ascontiguousarray(query[b].T).astype(MM_NP)
        xkT_b = np.ascontiguousarray(xk_g.T).astype(MM_NP)
        xvT_b = np.zeros((P * 7, NKV), MM_NP)
        xvT_b[:DIM] = xv_g.T
        xvT_b[DIM] = 1.0
        pb_b = np.where(np.arange(NKV) < cnt, 0.0, NEG).astype(np.float32)
        for g in range(2):
            sl = slice(DGRP * g, DGRP * (g + 1))
            in_maps.append({
                "xqT": xqT_b,
                "xkT": xkT_b,
                "xvT": xvT_b,
                "wqT": np.ascontiguousarray(q_w[sl].T).astype(MM_NP),
                "wkT": np.ascontiguousarray(k_w[sl].T).astype(MM_NP),
                "wvT": np.concatenate(
                    [v_w[sl].T, v_b[None, sl],
                     np.zeros((P - 1, DGRP), np.float32)], axis=0).astype(MM_NP),
                "woT": np.ascontiguousarray(o_w[:, sl].T).astype(MM_NP),
                "qb": q_b[sl].copy(),
                "kb": k_b[sl].copy(),
                "ob": o_b if g == 0 else zeros_ob,
                "pb": pb_b,
            })

    res = run_bass_kernel_spmd(nc, in_maps, core_ids=list(range(N_CORES)),
                               **_run_opts)
    global LAST_RES
    LAST_RES = res
    out = np.empty((BS, SEQ, DIM), np.float32)
    for b in range(BS):
        out[b] = res.results[2 * b]["out"] + res.results[2 * b + 1]["out"]
    return out


# revision 25
# speedup vs baseline: 1.3721x; 1.2581x over previous
"""Multi-head self-attention (B=4, S=2048, D=768, H=12, dh=64) on 8 trn2 cores.

Sharding: core = b*2 + g  (b = batch 0..3, g = head-group of 6 heads).
Each core computes q/k/v projections for its 6 heads over the full sequence,
masked softmax attention, and a partial output projection (column slice of
o_w => row-parallel). Host sums the two partial outputs per batch element.

Key points:
  - mask gather: only unmasked k positions (padded to a multiple of 128) are
    shipped/projected/exp'd; padding columns get a -1e30 per-partition bias
    inside the ACT exp instruction (out = exp(scale*s + bias)).
  - scoresT [k, q] layout so softmax weights feed the context matmul as lhsT
    with no transpose; softmax denominators come free from an appended
    ones-column in v (psum row 64 of the context matmul).
  - per-head normalization deferred: unnormalized ctx + sums copied to SBUF,
    one reciprocal op, PE rank-1 broadcast of recip across partitions, one
    tensor_tensor multiply per head.
  - biases: q/k bias = per-partition DVE tensor_scalar on psum eviction;
    v bias via contraction-augmentation (ones row in xvT, v_b row in wvT);
    o_b broadcast across partitions once via PE rank-1, added on psum evict
    (zeros passed for the g==1 cores so the host sum applies it once).
"""

import numpy as np
import ml_dtypes

import concourse.bass as bass
import concourse.mybir as mybir
import concourse.tile as tile
from concourse import bacc
from concourse.bass_utils import run_bass_kernel_spmd

BS, SEQ, DIM, NH = 4, 2048, 768, 12
DH = 64
HEADS = 6            # heads per core
DGRP = HEADS * DH    # 384
N_CORES = 8
P = 128

F32 = mybir.dt.float32
F32R = mybir.dt.float32r  # fp32 I/O, ~1.5e-4 matmul precision, 4x faster than fp32
BF16 = mybir.dt.bfloat16

# matmul dtype config (bf16 = 4x faster PE than fp32)
MM_DT = BF16
MM_NP = ml_dtypes.bfloat16 if MM_DT == BF16 else np.float32

NEG = -1.0e30


def _build(NKV: int):
    """Build the per-core Bass program, parameterized by padded kv length."""
    KC = NKV // P          # k chunks
    QC = SEQ // P          # 16
    NT = SEQ // 512        # 4 q-slices
    KIN = DIM // P         # 6 contraction chunks for q/k proj
    KIN_V = 7              # 768 inputs + ones row, padded to 896

    nc = bacc.Bacc(None, target_bir_lowering=False, debug=False)

    xqT = nc.declare_dram_parameter("xqT", [DIM, SEQ], MM_DT, isOutput=False)
    xkT = nc.declare_dram_parameter("xkT", [DIM, NKV], MM_DT, isOutput=False)
    xvT = nc.declare_dram_parameter("xvT", [P * KIN_V, NKV], MM_DT, isOutput=False)
    wqT = nc.declare_dram_parameter("wqT", [DIM, DGRP], MM_DT, isOutput=False)
    wkT = nc.declare_dram_parameter("wkT", [DIM, DGRP], MM_DT, isOutput=False)
    wvT = nc.declare_dram_parameter("wvT", [P * KIN_V, DGRP], MM_DT, isOutput=False)
    woT = nc.declare_dram_parameter("woT", [DGRP, DIM], MM_DT, isOutput=False)
    qb = nc.declare_dram_parameter("qb", [DGRP], F32, isOutput=False)
    kb = nc.declare_dram_parameter("kb", [DGRP], F32, isOutput=False)
    ob = nc.declare_dram_parameter("ob", [DIM], F32, isOutput=False)
    pb = nc.declare_dram_parameter("pb", [NKV], F32, isOutput=False)
    out = nc.declare_dram_parameter("out", [SEQ, DIM], F32, isOutput=True)

    xqT_r = xqT.rearrange("(kk pi) n -> pi kk n", pi=P)
    xkT_r = xkT.rearrange("(kk pi) n -> pi kk n", pi=P)
    xvT_r = xvT.rearrange("(kk pi) n -> pi kk n", pi=P)
    wqT_r = wqT.rearrange("(kk pi) n -> pi kk n", pi=P)
    wkT_r = wkT.rearrange("(kk pi) n -> pi kk n", pi=P)
    wvT_r = wvT.rearrange("(kk pi) n -> pi kk n", pi=P)
    woT_r = woT.rearrange("(kk pi) n -> pi kk n", pi=P)
    qb_r = qb.rearrange("(m pi) -> pi m", pi=P)
    kb_r = kb.rearrange("(m pi) -> pi m", pi=P)
    pb_r = pb.rearrange("(c pi) -> pi c", pi=P)

    with tile.TileContext(nc) as tc:
        with (
            tc.tile_pool(name="const", bufs=1) as const,
            tc.tile_pool(name="persist", bufs=1) as persist,
            tc.tile_pool(name="expp", bufs=2) as expp,
            tc.tile_pool(name="outp", bufs=2) as outp,
        ):
            # ---- constants ----
            pb_sb = const.tile([P, KC], F32)
            nc.sync.dma_start(pb_sb[:], pb_r)
            qb_sb = const.tile([P, 3], F32)
            nc.sync.dma_start(qb_sb[:], qb_r)
            kb_sb = const.tile([P, 3], F32)
            nc.sync.dma_start(kb_sb[:], kb_r)
            wo_sb = const.tile([P, 3, DIM], MM_DT)
            nc.sync.dma_start(wo_sb[:], woT_r)
            ones_sb = const.tile([1, P], F32)
            nc.vector.memset(ones_sb[:], 1.0)
            ob_row = const.tile([1, DIM], F32)
            nc.sync.dma_start(ob_row[:], ob[None, :])
            ob_bc = const.tile([P, DIM], F32)

            # ---- persistent activations ----
            qT_sb = persist.tile([P, 3, SEQ], MM_DT)
            kT_sb = persist.tile([P, 3, NKV], MM_DT)
            v_sb = persist.tile([P, KC, HEADS * 65], MM_DT)
            ctxu_sb = persist.tile([P, 3, SEQ], MM_DT)
            ctx_sb = persist.tile([P, 3, SEQ], MM_DT)

            # ones column per head in v (gives softmax sums in psum row 64)
            for h in range(HEADS):
                nc.vector.memset(v_sb[:, :, 65 * h + 64], 1.0)

            with tc.tile_pool(name="wpool", bufs=1) as wpool, \
                 tc.tile_pool(name="xslice", bufs=2) as xslice, \
                 tc.tile_pool(name="psA", bufs=2, space="PSUM") as psA:
                # o_b broadcast across partitions (rank-1 matmul)
                for n0, nsz in ((0, 512), (512, 256)):
                    ps = psA.tile([P, 512], F32, tag="psA0")
                    nc.tensor.matmul(ps[:, 0:nsz], ones_sb[:],
                                     ob_row[:, n0:n0 + nsz],
                                     start=True, stop=True)
                    nc.vector.tensor_copy(out=ob_bc[:, n0:n0 + nsz],
                                          in_=ps[:, 0:nsz])

                wq_sb = wpool.tile([P, KIN, DGRP], MM_DT)
                nc.sync.dma_start(wq_sb[:], wqT_r)
                wk_sb = wpool.tile([P, KIN, DGRP], MM_DT)
                nc.sync.dma_start(wk_sb[:], wkT_r)
                wv_sb = wpool.tile([P, KIN_V, DGRP], MM_DT)
                nc.sync.dma_start(wv_sb[:], wvT_r)

                # ---- q projection: qT[384, 2048] = wqT.T @ xqT (+qb) ----
                for nt in range(NT):
                    xq_t = xslice.tile([P, KIN, 512], MM_DT, tag="xq")
                    nc.sync.dma_start(xq_t[:], xqT_r[:, :, nt * 512:(nt + 1) * 512])
                    for m in range(3):
                        ps = psA.tile([P, 512], F32, tag=f"psA{(nt * 3 + m) % 2}")
                        for kk in range(KIN):
                            nc.tensor.matmul(
                                ps[:],
                                wq_sb[:, kk, m * P:(m + 1) * P],
                                xq_t[:, kk, :],
                                start=(kk == 0), stop=(kk == KIN - 1),
                            )
                        nc.vector.tensor_scalar_add(
                            qT_sb[:, m, nt * 512:(nt + 1) * 512], ps[:],
                            qb_sb[:, m, None],
                        )

                # ---- k projection: kT[384, NKV] ----
                ksl = []
                o = 0
                while o < NKV:
                    w = min(512, NKV - o)
                    ksl.append((o, w))
                    o += w
                for nt, (o0, w) in enumerate(ksl):
                    xk_t = xslice.tile([P, KIN, 512], MM_DT, tag="xk")
                    nc.sync.dma_start(xk_t[:, :, 0:w], xkT_r[:, :, o0:o0 + w])
                    for m in range(3):
                        ps = psA.tile([P, 512], F32, tag=f"psA{(nt * 3 + m) % 2}")
                        for kk in range(KIN):
                            nc.tensor.matmul(
                                ps[:, 0:w],
                                wk_sb[:, kk, m * P:(m + 1) * P],
                                xk_t[:, kk, 0:w],
                                start=(kk == 0), stop=(kk == KIN - 1),
                            )
                        nc.vector.tensor_scalar_add(
                            kT_sb[:, m, o0:o0 + w], ps[:, 0:w],
                            kb_sb[:, m, None],
                        )

                # ---- v projection: v[NKV, 384] (k rows on partitions) ----
                for m in range(KC):
                    xv_t = xslice.tile([P, KIN_V, P], MM_DT, tag="xv")
                    nc.sync.dma_start(xv_t[:], xvT_r[:, :, m * P:(m + 1) * P])
                    ps = psA.tile([P, DGRP], F32, tag=f"psA{m % 2}")
                    for kk in range(KIN_V):
                        nc.tensor.matmul(
                            ps[:],
                            xv_t[:, kk, :],
                            wv_sb[:, kk, :],
                            start=(kk == 0), stop=(kk == KIN_V - 1),
                        )
                    for h in range(HEADS):
                        nc.vector.tensor_copy(
                            out=v_sb[:, m, 65 * h:65 * h + 64],
                            in_=ps[:, 64 * h:64 * h + 64],
                        )

            # ---- attention, unit = (head, q-half) ----
            # [128,1024] psum tiles, double-buffered scores AND ctx (2+2+2+2
            # banks) so the PE stream stays dense across units (HAM warm)
            QH = 1024
            with tc.tile_pool(name="psS", bufs=2, space="PSUM") as psS, \
                 tc.tile_pool(name="psC", bufs=2, space="PSUM") as psC, \
                 tc.tile_pool(name="stat", bufs=2) as stat:
                for h in range(HEADS):
                    chunk, off = h // 2, 64 * (h % 2)
                    for qh in range(SEQ // QH):
                        q0 = qh * QH
                        ps_ctx = psC.tile([P, QH], F32, tag="ctx")
                        for kc in range(KC):
                            ps_s = psS.tile([P, QH], F32, tag="s")
                            for qt in range(QH // 512):
                                nc.tensor.matmul(
                                    ps_s[:, qt * 512:(qt + 1) * 512],
                                    kT_sb[off:off + DH, chunk, kc * P:(kc + 1) * P],
                                    qT_sb[off:off + DH, chunk,
                                          q0 + qt * 512:q0 + (qt + 1) * 512],
                                    start=True, stop=True,
                                )
                            exp_t = expp.tile([P, QH], MM_DT, tag="exp")
                            nc.scalar.activation(
                                exp_t[:], ps_s[:], mybir.ActivationFunctionType.Exp,
                                bias=pb_sb[:, kc, None], scale=0.125,
                            )
                            for qt in range(QH // 512):
                                nc.tensor.matmul(
                                    ps_ctx[0:65, qt * 512:(qt + 1) * 512],
                                    v_sb[:, kc, 65 * h:65 * h + 65],
                                    exp_t[:, qt * 512:(qt + 1) * 512],
                                    start=(kc == 0), stop=(kc == KC - 1),
                                )
                        # evict unnormalized ctx + sums, then normalize:
                        # broadcast 1/sums into rows 64..127 of the ctx psum
                        # tile (free once the sums row is copied out)
                        nc.vector.tensor_copy(
                            out=ctxu_sb[off:off + DH, chunk, q0:q0 + QH],
                            in_=ps_ctx[0:64, :],
                        )
                        sums_t = stat.tile([1, QH], F32, tag="sums")
                        nc.vector.tensor_copy(out=sums_t[:], in_=ps_ctx[64:65, :])
                        recip_t = stat.tile([1, QH], F32, tag="recip")
                        nc.vector.reciprocal_approx_fast(out=recip_t[:], in_=sums_t[:])
                        for qt in range(QH // 512):
                            nc.tensor.matmul(
                                ps_ctx[64:128, qt * 512:(qt + 1) * 512],
                                ones_sb[:, 0:DH],
                                recip_t[:, qt * 512:(qt + 1) * 512],
                                start=True, stop=True,
                            )
                        nc.vector.tensor_tensor(
                            ctx_sb[off:off + DH, chunk, q0:q0 + QH],
                            ctxu_sb[off:off + DH, chunk, q0:q0 + QH],
                            ps_ctx[64:128, :],
                            mybir.AluOpType.mult,
                        )

            # ---- output projection: out[2048, 768] (+ o_b) ----
            with tc.tile_pool(name="psO", bufs=2, space="PSUM") as psO:
                for qc in range(QC):
                    o_t = outp.tile([P, DIM], F32, tag="o")
                    ps = psO.tile([P, DIM], F32, tag=f"psO{qc % 2}")
                    for kk in range(3):
                        for n0, nsz in ((0, 512), (512, 256)):
                            nc.tensor.matmul(
                                ps[:, n0:n0 + nsz],
                                ctx_sb[:, kk, qc * P:(qc + 1) * P],
                                wo_sb[:, kk, n0:n0 + nsz],
                                start=(kk == 0), stop=(kk == 2),
                            )
                    nc.vector.tensor_tensor(
                        o_t[:], ps[:], ob_bc[:], mybir.AluOpType.add,
                    )
                    nc.sync.dma_start(out[qc * P:(qc + 1) * P, :], o_t[:])

    nc.compile()
    return nc


_cache: dict = {}

# test harnesses may set e.g. {"trace": True, "tmpdir": ...}; empty for grading
_run_opts: dict = {}
LAST_RES = None


def _get_nc(NKV: int):
    if NKV not in _cache:
        _cache[NKV] = _build(NKV)
    return _cache[NKV]


def kernel(query, key_, value, mask, q_w, q_b, k_w, k_b, v_w, v_b, o_w, o_b):
    query = np.asarray(query, np.float32)
    key_ = np.asarray(key_, np.float32)
    value = np.asarray(value, np.float32)
    mask = np.asarray(mask)
    q_w = np.asarray(q_w, np.float32)
    q_b = np.asarray(q_b, np.float32)
    k_w = np.asarray(k_w, np.float32)
    k_b = np.asarray(k_b, np.float32)
    v_w = np.asarray(v_w, np.float32)
    v_b = np.asarray(v_b, np.float32)
    o_w = np.asarray(o_w, np.float32)
    o_b = np.asarray(o_b, np.float32)

    counts = (mask != 0).sum(axis=1)
    NKV = max(P, int(-(-int(counts.max()) // P) * P))
    nc = _get_nc(NKV)

    zeros_ob = np.zeros_like(o_b)
    in_maps = []
    for b in range(BS):
        idx = np.nonzero(mask[b])[0]
        cnt = len(idx)
        xk_g = np.zeros((NKV, DIM), np.float32)
        xv_g = np.zeros((NKV, DIM), np.float32)
        xk_g[:cnt] = key_[b][idx]
        xv_g[:cnt] = value[b][idx]
        xqT_b = np.ascontiguousarray(query[b].T).astype(MM_NP)
        xkT_b = np.ascontiguousarray(xk_g.T).astype(MM_NP)
        xvT_b = np.zeros((P * 7, NKV), MM_NP)
        xvT_b[:DIM] = xv_g.T
        xvT_b[DIM] = 1.0
        pb_b = np.where(np.arange(NKV) < cnt, 0.0, NEG).astype(np.float32)
        for g in range(2):
            sl = slice(DGRP * g, DGRP * (g + 1))
            in_maps.append({
                "xqT": xqT_b,
                "xkT": xkT_b,
                "xvT": xvT_b,
                "wqT": np.ascontiguousarray(q_w[sl].T).astype(MM_NP),
                "wkT": np.ascontiguousarray(k_w[sl].T).astype(MM_NP),
                "wvT": np.concatenate(
                    [v_w[sl].T, v_b[None, sl],
                     np.zeros((P - 1, DGRP), np.float32)], axis=0).astype(MM_NP),
                "woT": np.ascontiguousarray(o_w[:, sl].T).astype(MM_NP),
                "qb": q_b[sl].copy(),
                "kb": k_b[sl].copy(),
                "ob": o_b if g == 0 else zeros_ob,
                "pb": pb_b,
            })

    res = run_bass_kernel_spmd(nc, in_maps, core_ids=list(range(N_CORES)),
                               **_run_opts)
    global LAST_RES
    LAST_RES = res
    out = np.empty((BS, SEQ, DIM), np.float32)
    for b in range(BS):
        out[b] = res.results[2 * b]["out"] + res.results[2 * b + 1]["out"]
    return out


# revision 26
# speedup vs baseline: 1.6215x; 1.1818x over previous
"""Multi-head self-attention (B=4, S=2048, D=768, H=12, dh=64) on 8 trn2 cores.

Sharding: core = b*2 + g  (b = batch 0..3, g = head-group of 6 heads).
Each core computes q/k/v projections for its 6 heads over the full sequence,
masked softmax attention, and a partial output projection (column slice of
o_w => row-parallel). Host sums the two partial outputs per batch element.

Key points:
  - mask gather: only unmasked k positions (padded to a multiple of 128) are
    shipped/projected/exp'd; padding columns get a -1e30 per-partition bias
    inside the ACT exp instruction (out = exp(scale*s + bias)).
  - scoresT [k, q] layout so softmax weights feed the context matmul as lhsT
    with no transpose; softmax denominators come free from an appended
    ones-column in v (psum row 64 of the context matmul).
  - ALL matmuls use a full K=128 contraction: the PE HAM clock gate only
    un-throttles (1.2 -> 2.4 GHz) for high row-utilization streams, so the
    64-dim per-head score contractions are zero-padded to 128 rows (kTz holds
    each head's kT in its own 64-row half, other half zero), and the rank-1
    recip/bias broadcasts use a [128, M] ones-row matrix against an input
    whose rows 1..127 are zeroed.
  - per-head-per-qhalf normalization: sums row evicted, approx reciprocal
    (~18-bit, 5x faster than exact), broadcast into rows 64..127 of the ctx
    psum tile via the K=128 ones-row matmul, one tensor_tensor multiply.
  - biases: q/k bias = per-partition DVE tensor_scalar on psum eviction;
    v bias via contraction-augmentation (ones row in xvT, v_b row in wvT);
    o_b broadcast across partitions once, added on psum evict (zeros passed
    for the g==1 cores so the host sum applies it once).
"""

import numpy as np
import ml_dtypes

import concourse.bass as bass
import concourse.mybir as mybir
import concourse.tile as tile
from concourse import bacc
from concourse.bass_utils import run_bass_kernel_spmd

BS, SEQ, DIM, NH = 4, 2048, 768, 12
DH = 64
HEADS = 6            # heads per core
DGRP = HEADS * DH    # 384
N_CORES = 8
P = 128
QH = 1024            # q-half width in the attention loop

F32 = mybir.dt.float32
BF16 = mybir.dt.bfloat16

MM_DT = BF16
MM_NP = ml_dtypes.bfloat16 if MM_DT == BF16 else np.float32

NEG = -1.0e30


def _build(NKV: int):
    """Build the per-core Bass program, parameterized by padded kv length."""
    KC = NKV // P          # k chunks
    QC = SEQ // P          # 16
    NT = SEQ // 512        # 4
    KIN = DIM // P         # 6 contraction chunks for q/k proj
    KIN_V = 7              # 768 inputs + ones row, padded to 896

    nc = bacc.Bacc(None, target_bir_lowering=False, debug=False)

    xqT = nc.declare_dram_parameter("xqT", [DIM, SEQ], MM_DT, isOutput=False)
    xkT = nc.declare_dram_parameter("xkT", [DIM, NKV], MM_DT, isOutput=False)
    xvT = nc.declare_dram_parameter("xvT", [P * KIN_V, NKV], MM_DT, isOutput=False)
    wqT = nc.declare_dram_parameter("wqT", [DIM, DGRP], MM_DT, isOutput=False)
    wkT = nc.declare_dram_parameter("wkT", [DIM, DGRP], MM_DT, isOutput=False)
    wvT = nc.declare_dram_parameter("wvT", [P * KIN_V, DGRP], MM_DT, isOutput=False)
    woT = nc.declare_dram_parameter("woT", [DGRP, DIM], MM_DT, isOutput=False)
    qb = nc.declare_dram_parameter("qb", [DGRP], F32, isOutput=False)
    kb = nc.declare_dram_parameter("kb", [DGRP], F32, isOutput=False)
    ob = nc.declare_dram_parameter("ob", [DIM], F32, isOutput=False)
    pb = nc.declare_dram_parameter("pb", [NKV], F32, isOutput=False)
    out = nc.declare_dram_parameter("out", [SEQ, DIM], F32, isOutput=True)

    xqT_r = xqT.rearrange("(kk pi) n -> pi kk n", pi=P)
    xkT_r = xkT.rearrange("(kk pi) n -> pi kk n", pi=P)
    xvT_r = xvT.rearrange("(kk pi) n -> pi kk n", pi=P)
    wqT_r = wqT.rearrange("(kk pi) n -> pi kk n", pi=P)
    wkT_r = wkT.rearrange("(kk pi) n -> pi kk n", pi=P)
    wvT_r = wvT.rearrange("(kk pi) n -> pi kk n", pi=P)
    woT_r = woT.rearrange("(kk pi) n -> pi kk n", pi=P)
    qb_r = qb.rearrange("(m pi) -> pi m", pi=P)
    kb_r = kb.rearrange("(m pi) -> pi m", pi=P)
    pb_r = pb.rearrange("(c pi) -> pi c", pi=P)

    with tile.TileContext(nc) as tc:
        with (
            tc.tile_pool(name="const", bufs=1) as const,
            tc.tile_pool(name="persist", bufs=1) as persist,
            tc.tile_pool(name="expp", bufs=2) as expp,
            tc.tile_pool(name="outp", bufs=2) as outp,
        ):
            # ---- constants ----
            pb_sb = const.tile([P, KC], F32)
            nc.sync.dma_start(pb_sb[:], pb_r)
            qb_sb = const.tile([P, 3], F32)
            nc.sync.dma_start(qb_sb[:], qb_r)
            kb_sb = const.tile([P, 3], F32)
            nc.sync.dma_start(kb_sb[:], kb_r)
            wo_sb = const.tile([P, 3, DIM], MM_DT)
            nc.sync.dma_start(wo_sb[:], woT_r)
            # ones-row matrix: row 0 all-ones, rows 1..127 zero. As lhsT this
            # replicates row 0 of the rhs into all M output partitions with a
            # full K=128 contraction (keeps the PE HAM clock warm).
            ones2_sb = const.tile([P, P], F32)
            nc.vector.memset(ones2_sb[:], 0.0)
            nc.vector.memset(ones2_sb[0:1, :], 1.0)
            ob_row = const.tile([P, DIM], F32)
            nc.vector.memset(ob_row[:], 0.0)
            nc.sync.dma_start(ob_row[0:1, :], ob[None, :])
            ob_bc = const.tile([P, DIM], F32)

            # ---- persistent activations ----
            qT_sb = persist.tile([P, 3, SEQ], MM_DT)
            # kTz: per head h, half 64*(h%2) holds kT_h, other half zero
            kTz_sb = persist.tile([P, HEADS, NKV], MM_DT)
            v_sb = persist.tile([P, KC, HEADS * 65], MM_DT)
            ctxu_sb = persist.tile([P, 3, SEQ], MM_DT)
            ctx_sb = persist.tile([P, 3, SEQ], MM_DT)
            # recip rows-zeroed tiles (row 0 = 1/sums, rows 1..127 = 0)
            recipA = persist.tile([P, QH], F32)
            recipB = persist.tile([P, QH], F32)

            nc.vector.memset(kTz_sb[:], 0.0)
            nc.vector.memset(recipA[:], 0.0)
            nc.vector.memset(recipB[:], 0.0)
            # ones column per head in v (gives softmax sums in psum row 64)
            for h in range(HEADS):
                nc.vector.memset(v_sb[:, :, 65 * h + 64], 1.0)

            with tc.tile_pool(name="wpool", bufs=1) as wpool, \
                 tc.tile_pool(name="xslice", bufs=2) as xslice, \
                 tc.tile_pool(name="psA", bufs=2, space="PSUM") as psA:
                # o_b broadcast across partitions
                for n0, nsz in ((0, 512), (512, 256)):
                    ps = psA.tile([P, 512], F32, tag="psA0")
                    nc.tensor.matmul(ps[:, 0:nsz], ones2_sb[:],
                                     ob_row[:, n0:n0 + nsz],
                                     start=True, stop=True)
                    nc.vector.tensor_copy(out=ob_bc[:, n0:n0 + nsz],
                                          in_=ps[:, 0:nsz])

                wq_sb = wpool.tile([P, KIN, DGRP], MM_DT)
                nc.sync.dma_start(wq_sb[:], wqT_r)
                wk_sb = wpool.tile([P, KIN, DGRP], MM_DT)
                nc.sync.dma_start(wk_sb[:], wkT_r)
                wv_sb = wpool.tile([P, KIN_V, DGRP], MM_DT)
                nc.sync.dma_start(wv_sb[:], wvT_r)

                # ---- q projection: qT[384, 2048] = wqT.T @ xqT (+qb) ----
                for nt in range(NT):
                    xq_t = xslice.tile([P, KIN, 512], MM_DT, tag="xq")
                    nc.sync.dma_start(xq_t[:], xqT_r[:, :, nt * 512:(nt + 1) * 512])
                    for m in range(3):
                        ps = psA.tile([P, 512], F32, tag=f"psA{(nt * 3 + m) % 2}")
                        for kk in range(KIN):
                            nc.tensor.matmul(
                                ps[:],
                                wq_sb[:, kk, m * P:(m + 1) * P],
                                xq_t[:, kk, :],
                                start=(kk == 0), stop=(kk == KIN - 1),
                            )
                        nc.vector.tensor_scalar_add(
                            qT_sb[:, m, nt * 512:(nt + 1) * 512], ps[:],
                            qb_sb[:, m, None],
                        )

                # ---- k projection into kTz (per-head 64-row halves) ----
                ksl = []
                o = 0
                while o < NKV:
                    w = min(512, NKV - o)
                    ksl.append((o, w))
                    o += w
                for nt, (o0, w) in enumerate(ksl):
                    xk_t = xslice.tile([P, KIN, 512], MM_DT, tag="xk")
                    nc.sync.dma_start(xk_t[:, :, 0:w], xkT_r[:, :, o0:o0 + w])
                    for m in range(3):
                        ps = psA.tile([P, 512], F32, tag=f"psA{(nt * 3 + m) % 2}")
                        for kk in range(KIN):
                            nc.tensor.matmul(
                                ps[:, 0:w],
                                wk_sb[:, kk, m * P:(m + 1) * P],
                                xk_t[:, kk, 0:w],
                                start=(kk == 0), stop=(kk == KIN - 1),
                            )
                        nc.vector.tensor_scalar_add(
                            kTz_sb[0:64, 2 * m, o0:o0 + w], ps[0:64, 0:w],
                            kb_sb[0:64, m, None],
                        )
                        nc.vector.tensor_scalar_add(
                            kTz_sb[64:128, 2 * m + 1, o0:o0 + w], ps[64:128, 0:w],
                            kb_sb[64:128, m, None],
                        )

                # ---- v projection: v[NKV, 384] (k rows on partitions) ----
                for m in range(KC):
                    xv_t = xslice.tile([P, KIN_V, P], MM_DT, tag="xv")
                    nc.sync.dma_start(xv_t[:], xvT_r[:, :, m * P:(m + 1) * P])
                    ps = psA.tile([P, DGRP], F32, tag=f"psA{m % 2}")
                    for kk in range(KIN_V):
                        nc.tensor.matmul(
                            ps[:],
                            xv_t[:, kk, :],
                            wv_sb[:, kk, :],
                            start=(kk == 0), stop=(kk == KIN_V - 1),
                        )
                    for h in range(HEADS):
                        nc.vector.tensor_copy(
                            out=v_sb[:, m, 65 * h:65 * h + 64],
                            in_=ps[:, 64 * h:64 * h + 64],
                        )

            # ---- attention, unit = (head, q-half) ----
            with tc.tile_pool(name="psS", bufs=2, space="PSUM") as psS, \
                 tc.tile_pool(name="psC", bufs=2, space="PSUM") as psC, \
                 tc.tile_pool(name="stat", bufs=2) as stat:
                for u, (h, qh) in enumerate(
                        [(h, qh) for h in range(HEADS) for qh in range(SEQ // QH)]):
                    chunk, off = h // 2, 64 * (h % 2)
                    q0 = qh * QH
                    recip_t = recipA if u % 2 == 0 else recipB
                    ps_ctx = psC.tile([P, QH], F32, tag="ctx")
                    for kc in range(KC):
                        ps_s = psS.tile([P, QH], F32, tag="s")
                        for qt in range(QH // 512):
                            nc.tensor.matmul(
                                ps_s[:, qt * 512:(qt + 1) * 512],
                                kTz_sb[:, h, kc * P:(kc + 1) * P],
                                qT_sb[:, chunk,
                                      q0 + qt * 512:q0 + (qt + 1) * 512],
                                start=True, stop=True,
                            )
                        exp_t = expp.tile([P, QH], MM_DT, tag="exp")
                        nc.scalar.activation(
                            exp_t[:], ps_s[:], mybir.ActivationFunctionType.Exp,
                            bias=pb_sb[:, kc, None], scale=0.125,
                        )
                        for qt in range(QH // 512):
                            nc.tensor.matmul(
                                ps_ctx[0:65, qt * 512:(qt + 1) * 512],
                                v_sb[:, kc, 65 * h:65 * h + 65],
                                exp_t[:, qt * 512:(qt + 1) * 512],
                                start=(kc == 0), stop=(kc == KC - 1),
                            )
                    # evict unnormalized ctx + sums; normalize via recip
                    # broadcast into rows 64..127 of the ctx psum tile
                    nc.vector.tensor_copy(
                        out=ctxu_sb[off:off + DH, chunk, q0:q0 + QH],
                        in_=ps_ctx[0:64, :],
                    )
                    sums_t = stat.tile([1, QH], F32, tag="sums")
                    nc.vector.tensor_copy(out=sums_t[:], in_=ps_ctx[64:65, :])
                    nc.vector.reciprocal_approx_fast(
                        out=recip_t[0:1, :], in_=sums_t[:])
                    for qt in range(QH // 512):
                        nc.tensor.matmul(
                            ps_ctx[64:128, qt * 512:(qt + 1) * 512],
                            ones2_sb[:, 0:DH],
                            recip_t[:, qt * 512:(qt + 1) * 512],
                            start=True, stop=True,
                        )
                    nc.vector.tensor_tensor(
                        ctx_sb[off:off + DH, chunk, q0:q0 + QH],
                        ctxu_sb[off:off + DH, chunk, q0:q0 + QH],
                        ps_ctx[64:128, :],
                        mybir.AluOpType.mult,
                    )

            # ---- output projection: out[2048, 768] (+ o_b) ----
            with tc.tile_pool(name="psO", bufs=2, space="PSUM") as psO:
                for qc in range(QC):
                    o_t = outp.tile([P, DIM], F32, tag="o")
                    ps = psO.tile([P, DIM], F32, tag=f"psO{qc % 2}")
                    for kk in range(3):
                        for n0, nsz in ((0, 512), (512, 256)):
                            nc.tensor.matmul(
                                ps[:, n0:n0 + nsz],
                                ctx_sb[:, kk, qc * P:(qc + 1) * P],
                                wo_sb[:, kk, n0:n0 + nsz],
                                start=(kk == 0), stop=(kk == 2),
                            )
                    nc.vector.tensor_tensor(
                        o_t[:], ps[:], ob_bc[:], mybir.AluOpType.add,
                    )
                    nc.sync.dma_start(out[qc * P:(qc + 1) * P, :], o_t[:])

    nc.compile()
    return nc


_cache: dict = {}

# test harnesses may set e.g. {"trace": True, "tmpdir": ...}; empty for grading
_run_opts: dict = {}
LAST_RES = None


def _get_nc(NKV: int):
    if NKV not in _cache:
        _cache[NKV] = _build(NKV)
    return _cache[NKV]


def kernel(query, key_, value, mask, q_w, q_b, k_w, k_b, v_w, v_b, o_w, o_b):
    query = np.asarray(query, np.float32)
    key_ = np.asarray(key_, np.float32)
    value = np.asarray(value, np.float32)
    mask = np.asarray(mask)
    q_w = np.asarray(q_w, np.float32)
    q_b = np.asarray(q_b, np.float32)
    k_w = np.asarray(k_w, np.float32)
    k_b = np.asarray(k_b, np.float32)
    v_w = np.asarray(v_w, np.float32)
    v_b = np.asarray(v_b, np.float32)
    o_w = np.asarray(o_w, np.float32)
    o_b = np.asarray(o_b, np.float32)

    counts = (mask != 0).sum(axis=1)
    NKV = max(P, int(-(-int(counts.max()) // P) * P))
    nc = _get_nc(NKV)

    zeros_ob = np.zeros_like(o_b)
    in_maps = []
    for b in range(BS):
        idx = np.nonzero(mask[b])[0]
        cnt = len(idx)
        xk_g = np.zeros((NKV, DIM), np.float32)
        xv_g = np.zeros((NKV, DIM), np.float32)
        xk_g[:cnt] = key_[b][idx]
        xv_g[:cnt] = value[b][idx]
        xqT_b = np.ascontiguousarray(query[b].T).astype(MM_NP)
        xkT_b = np.ascontiguousarray(xk_g.T).astype(MM_NP)
        xvT_b = np.zeros((P * 7, NKV), MM_NP)
        xvT_b[:DIM] = xv_g.T
        xvT_b[DIM] = 1.0
        pb_b = np.where(np.arange(NKV) < cnt, 0.0, NEG).astype(np.float32)
        for g in range(2):
            sl = slice(DGRP * g, DGRP * (g + 1))
            in_maps.append({
                "xqT": xqT_b,
                "xkT": xkT_b,
                "xvT": xvT_b,
                "wqT": np.ascontiguousarray(q_w[sl].T).astype(MM_NP),
                "wkT": np.ascontiguousarray(k_w[sl].T).astype(MM_NP),
                "wvT": np.concatenate(
                    [v_w[sl].T, v_b[None, sl],
                     np.zeros((P - 1, DGRP), np.float32)], axis=0).astype(MM_NP),
                "woT": np.ascontiguousarray(o_w[:, sl].T).astype(MM_NP),
                "qb": q_b[sl].copy(),
                "kb": k_b[sl].copy(),
                "ob": o_b if g == 0 else zeros_ob,
                "pb": pb_b,
            })

    res = run_bass_kernel_spmd(nc, in_maps, core_ids=list(range(N_CORES)),
                               **_run_opts)
    global LAST_RES
    LAST_RES = res
    out = np.empty((BS, SEQ, DIM), np.float32)
    for b in range(BS):
        out[b] = res.results[2 * b]["out"] + res.results[2 * b + 1]["out"]
    return out


# revision 29
# speedup vs baseline: 1.9700x; 1.2149x over previous
"""Multi-head self-attention (B=4, S=2048, D=768, H=12, dh=64) on 8 trn2 cores.

Sharding: core = b*2 + g  (b = batch 0..3, g = head-group of 6 heads).
Each core computes q/k/v projections for its 6 heads over the full sequence,
masked softmax attention, and a partial output projection (column slice of
o_w => row-parallel). Host sums the two partial outputs per batch element.

Key points:
  - mask gather: only unmasked k positions (padded to a multiple of 128) are
    shipped/projected/exp'd; padding columns get a -1e30 per-partition bias
    inside the ACT exp instruction (out = exp(scale*s + bias)).
  - scoresT [k, q] layout so softmax weights feed the context matmul as lhsT
    with no transpose; softmax denominators come free from an appended
    ones-column in v (psum row 64 of the context matmul).
  - ALL matmuls use a full K=128 contraction: the PE HAM clock gate only
    un-throttles (1.2 -> 2.4 GHz) for high row-utilization streams, so the
    64-dim per-head score contractions are zero-padded to 128 rows (kTz holds
    each head's kT in its own 64-row half, other half zero), and the rank-1
    recip/bias broadcasts use a [128, M] ones-row matrix against an input
    whose rows 1..127 are zeroed.
  - per-head-per-qhalf normalization: sums row evicted, approx reciprocal
    (~18-bit, 5x faster than exact), broadcast into rows 64..127 of the ctx
    psum tile via the K=128 ones-row matmul, one tensor_tensor multiply.
  - biases: q/k bias = per-partition DVE tensor_scalar on psum eviction;
    v bias via contraction-augmentation (ones row in xvT, v_b row in wvT);
    o_b broadcast across partitions once, added on psum evict (zeros passed
    for the g==1 cores so the host sum applies it once).
"""

import numpy as np
import ml_dtypes

import concourse.bass as bass
import concourse.mybir as mybir
import concourse.tile as tile
from concourse import bacc
from concourse.bass_utils import run_bass_kernel_spmd

BS, SEQ, DIM, NH = 4, 2048, 768, 12
DH = 64
HEADS = 6            # heads per core
DGRP = HEADS * DH    # 384
N_CORES = 8
P = 128
QH = 1024            # q-half width in the attention loop

F32 = mybir.dt.float32
BF16 = mybir.dt.bfloat16

MM_DT = BF16
MM_NP = ml_dtypes.bfloat16 if MM_DT == BF16 else np.float32

NEG = -1.0e30


def _build(NKV: int):
    """Build the per-core Bass program, parameterized by padded kv length."""
    KC = NKV // P          # k chunks
    QC = SEQ // P          # 16
    NT = SEQ // 512        # 4
    KIN = DIM // P         # 6 contraction chunks for q/k proj
    KIN_V = 7              # 768 inputs + ones row, padded to 896

    nc = bacc.Bacc(None, target_bir_lowering=False, debug=False)

    xqT = nc.declare_dram_parameter("xqT", [DIM, SEQ], MM_DT, isOutput=False)
    xkT = nc.declare_dram_parameter("xkT", [DIM, NKV], MM_DT, isOutput=False)
    xvT = nc.declare_dram_parameter("xvT", [P * KIN_V, NKV], MM_DT, isOutput=False)
    wqT = nc.declare_dram_parameter("wqT", [DIM, DGRP], MM_DT, isOutput=False)
    wkT = nc.declare_dram_parameter("wkT", [DIM, DGRP], MM_DT, isOutput=False)
    wvT = nc.declare_dram_parameter("wvT", [P * KIN_V, DGRP], MM_DT, isOutput=False)
    woT = nc.declare_dram_parameter("woT", [DGRP, DIM], MM_DT, isOutput=False)
    qb = nc.declare_dram_parameter("qb", [DGRP], F32, isOutput=False)
    kb = nc.declare_dram_parameter("kb", [DGRP], F32, isOutput=False)
    ob = nc.declare_dram_parameter("ob", [DIM], F32, isOutput=False)
    pb = nc.declare_dram_parameter("pb", [NKV], F32, isOutput=False)
    out = nc.declare_dram_parameter("out", [SEQ, DIM], F32, isOutput=True)

    xqT_r = xqT.rearrange("(kk pi) n -> pi kk n", pi=P)
    xkT_r = xkT.rearrange("(kk pi) n -> pi kk n", pi=P)
    xvT_r = xvT.rearrange("(kk pi) n -> pi kk n", pi=P)
    wqT_r = wqT.rearrange("(kk pi) n -> pi kk n", pi=P)
    wkT_r = wkT.rearrange("(kk pi) n -> pi kk n", pi=P)
    wvT_r = wvT.rearrange("(kk pi) n -> pi kk n", pi=P)
    woT_r = woT.rearrange("(kk pi) n -> pi kk n", pi=P)
    qb_r = qb.rearrange("(m pi) -> pi m", pi=P)
    kb_r = kb.rearrange("(m pi) -> pi m", pi=P)
    pb_r = pb.rearrange("(c pi) -> pi c", pi=P)

    with tile.TileContext(nc) as tc:
        with (
            tc.tile_pool(name="const", bufs=1) as const,
            tc.tile_pool(name="persist", bufs=1) as persist,
            tc.tile_pool(name="expp", bufs=3) as expp,
            tc.tile_pool(name="outp", bufs=3) as outp,
        ):
            # ---- constants ----
            pb_sb = const.tile([P, KC], F32)
            nc.sync.dma_start(pb_sb[:], pb_r)
            qb_sb = const.tile([P, 3], F32)
            nc.sync.dma_start(qb_sb[:], qb_r)
            kb_sb = const.tile([P, 3], F32)
            nc.sync.dma_start(kb_sb[:], kb_r)
            wo_sb = const.tile([P, 3, DIM], MM_DT)
            nc.sync.dma_start(wo_sb[:], woT_r)
            # ones-row matrix: row 0 all-ones, rows 1..127 zero. As lhsT this
            # replicates row 0 of the rhs into all M output partitions with a
            # full K=128 contraction (keeps the PE HAM clock warm).
            ones2_sb = const.tile([P, P], F32)
            nc.vector.memset(ones2_sb[:], 0.0)
            nc.vector.memset(ones2_sb[0:1, :], 1.0)
            ob_row = const.tile([P, DIM], F32)
            nc.vector.memset(ob_row[:], 0.0)
            nc.sync.dma_start(ob_row[0:1, :], ob[None, :])
            ob_bc = const.tile([P, DIM], F32)

            # ---- persistent activations ----
            qT_sb = persist.tile([P, 3, SEQ], MM_DT)
            # kTz: per head h, half 64*(h%2) holds kT_h, other half zero
            kTz_sb = persist.tile([P, HEADS, NKV], MM_DT)
            v_sb = persist.tile([P, KC, HEADS * 65], MM_DT)
            ctx_sb = persist.tile([P, 3, SEQ], MM_DT)
            # recip rows-zeroed tiles (row 0 = 1/sums, rows 1..127 = 0)
            recipA = persist.tile([P, QH], F32)
            recipB = persist.tile([P, QH], F32)

            nc.vector.memset(kTz_sb[:], 0.0)
            nc.vector.memset(recipA[:], 0.0)
            nc.vector.memset(recipB[:], 0.0)
            # ones column per head in v (gives softmax sums in psum row 64)
            for h in range(HEADS):
                nc.vector.memset(v_sb[:, :, 65 * h + 64], 1.0)

            with tc.tile_pool(name="wpool", bufs=1) as wpool, \
                 tc.tile_pool(name="xslice", bufs=3) as xslice, \
                 tc.tile_pool(name="psA", bufs=2, space="PSUM") as psA:
                # o_b broadcast across partitions
                for n0, nsz in ((0, 512), (512, 256)):
                    ps = psA.tile([P, 512], F32, tag="psA0")
                    nc.tensor.matmul(ps[:, 0:nsz], ones2_sb[:],
                                     ob_row[:, n0:n0 + nsz],
                                     start=True, stop=True)
                    nc.vector.tensor_copy(out=ob_bc[:, n0:n0 + nsz],
                                          in_=ps[:, 0:nsz])

                wq_sb = wpool.tile([P, KIN, DGRP], MM_DT)
                nc.sync.dma_start(wq_sb[:], wqT_r)
                wk_sb = wpool.tile([P, KIN, DGRP], MM_DT)
                nc.sync.dma_start(wk_sb[:], wkT_r)
                wv_sb = wpool.tile([P, KIN_V, DGRP], MM_DT)
                nc.sync.dma_start(wv_sb[:], wvT_r)

                # ---- q projection: qT[384, 2048] = wqT.T @ xqT (+qb) ----
                for nt in range(NT):
                    xq_t = xslice.tile([P, KIN, 512], MM_DT, tag="xq")
                    nc.sync.dma_start(xq_t[:], xqT_r[:, :, nt * 512:(nt + 1) * 512])
                    for m in range(3):
                        ps = psA.tile([P, 512], F32, tag=f"psA{(nt * 3 + m) % 2}")
                        for kk in range(KIN):
                            nc.tensor.matmul(
                                ps[:],
                                wq_sb[:, kk, m * P:(m + 1) * P],
                                xq_t[:, kk, :],
                                start=(kk == 0), stop=(kk == KIN - 1),
                            )
                        nc.vector.tensor_scalar_add(
                            qT_sb[:, m, nt * 512:(nt + 1) * 512], ps[:],
                            qb_sb[:, m, None],
                        )

                # ---- k projection into kTz (per-head 64-row halves) ----
                ksl = []
                o = 0
                while o < NKV:
                    w = min(512, NKV - o)
                    ksl.append((o, w))
                    o += w
                for nt, (o0, w) in enumerate(ksl):
                    xk_t = xslice.tile([P, KIN, 512], MM_DT, tag="xk")
                    nc.sync.dma_start(xk_t[:, :, 0:w], xkT_r[:, :, o0:o0 + w])
                    for m in range(3):
                        ps = psA.tile([P, 512], F32, tag=f"psA{(nt * 3 + m) % 2}")
                        for kk in range(KIN):
                            nc.tensor.matmul(
                                ps[:, 0:w],
                                wk_sb[:, kk, m * P:(m + 1) * P],
                                xk_t[:, kk, 0:w],
                                start=(kk == 0), stop=(kk == KIN - 1),
                            )
                        nc.vector.tensor_scalar_add(
                            kTz_sb[0:64, 2 * m, o0:o0 + w], ps[0:64, 0:w],
                            kb_sb[0:64, m, None],
                        )
                        nc.vector.tensor_scalar_add(
                            kTz_sb[64:128, 2 * m + 1, o0:o0 + w], ps[64:128, 0:w],
                            kb_sb[64:128, m, None],
                        )

                # ---- v projection: v[NKV, 384] (k rows on partitions) ----
                for m in range(KC):
                    xv_t = xslice.tile([P, KIN_V, P], MM_DT, tag="xv")
                    nc.sync.dma_start(xv_t[:], xvT_r[:, :, m * P:(m + 1) * P])
                    ps = psA.tile([P, DGRP], F32, tag=f"psA{m % 2}")
                    for kk in range(KIN_V):
                        nc.tensor.matmul(
                            ps[:],
                            xv_t[:, kk, :],
                            wv_sb[:, kk, :],
                            start=(kk == 0), stop=(kk == KIN_V - 1),
                        )
                    for h in range(HEADS):
                        nc.vector.tensor_copy(
                            out=v_sb[:, m, 65 * h:65 * h + 64],
                            in_=ps[:, 64 * h:64 * h + 64],
                        )

            # ---- attention, unit = (head, q-half) ----
            with tc.tile_pool(name="psS", bufs=2, space="PSUM") as psS, \
                 tc.tile_pool(name="psC", bufs=2, space="PSUM") as psC, \
                 tc.tile_pool(name="stat", bufs=2) as stat:
                for u, (h, qh) in enumerate(
                        [(h, qh) for h in range(HEADS) for qh in range(SEQ // QH)]):
                    chunk, off = h // 2, 64 * (h % 2)
                    q0 = qh * QH
                    recip_t = recipA if u % 2 == 0 else recipB
                    ps_ctx = psC.tile([P, QH], F32, tag="ctx")
                    for kc in range(KC):
                        ps_s = psS.tile([P, QH], F32, tag="s")
                        for qt in range(QH // 512):
                            nc.tensor.matmul(
                                ps_s[:, qt * 512:(qt + 1) * 512],
                                kTz_sb[:, h, kc * P:(kc + 1) * P],
                                qT_sb[:, chunk,
                                      q0 + qt * 512:q0 + (qt + 1) * 512],
                                start=True, stop=True,
                            )
                        exp_t = expp.tile([P, QH], MM_DT, tag="exp")
                        nc.scalar.activation(
                            exp_t[:], ps_s[:], mybir.ActivationFunctionType.Exp,
                            bias=pb_sb[:, kc, None], scale=0.125,
                        )
                        for qt in range(QH // 512):
                            nc.tensor.matmul(
                                ps_ctx[0:65, qt * 512:(qt + 1) * 512],
                                v_sb[:, kc, 65 * h:65 * h + 65],
                                exp_t[:, qt * 512:(qt + 1) * 512],
                                start=(kc == 0), stop=(kc == KC - 1),
                            )
                    # normalize: recip of the sums row (read straight from
                    # psum), broadcast into rows 64..127 of the ctx psum tile,
                    # then multiply against the evicted unnormalized ctx
                    sums_t = stat.tile([1, QH], F32, tag="sums")
                    nc.vector.tensor_copy(out=sums_t[:], in_=ps_ctx[64:65, :])
                    nc.vector.reciprocal_approx_fast(
                        out=recip_t[0:1, :], in_=sums_t[:])
                    ctxu_t = stat.tile([DH, QH], MM_DT, tag="ctxu")
                    nc.vector.tensor_copy(out=ctxu_t[:], in_=ps_ctx[0:64, :])
                    for qt in range(QH // 512):
                        nc.tensor.matmul(
                            ps_ctx[64:128, qt * 512:(qt + 1) * 512],
                            ones2_sb[:, 0:DH],
                            recip_t[:, qt * 512:(qt + 1) * 512],
                            start=True, stop=True,
                        )
                    nc.vector.tensor_tensor(
                        ctx_sb[off:off + DH, chunk, q0:q0 + QH],
                        ctxu_t[:],
                        ps_ctx[64:128, :],
                        mybir.AluOpType.mult,
                    )

            # ---- output projection: out[2048, 768] (+ o_b) ----
            with tc.tile_pool(name="psO", bufs=2, space="PSUM") as psO:
                for qc in range(QC):
                    o_t = outp.tile([P, DIM], F32, tag="o")
                    ps = psO.tile([P, DIM], F32, tag=f"psO{qc % 2}")
                    for kk in range(3):
                        for n0, nsz in ((0, 512), (512, 256)):
                            nc.tensor.matmul(
                                ps[:, n0:n0 + nsz],
                                ctx_sb[:, kk, qc * P:(qc + 1) * P],
                                wo_sb[:, kk, n0:n0 + nsz],
                                start=(kk == 0), stop=(kk == 2),
                            )
                    nc.vector.tensor_tensor(
                        o_t[:], ps[:], ob_bc[:], mybir.AluOpType.add,
                    )
                    nc.sync.dma_start(out[qc * P:(qc + 1) * P, :], o_t[:])

    nc.compile()
    return nc


_cache: dict = {}

# test harnesses may set e.g. {"trace": True, "tmpdir": ...}; empty for grading
_run_opts: dict = {}
LAST_RES = None


def _get_nc(NKV: int):
    if NKV not in _cache:
        _cache[NKV] = _build(NKV)
    return _cache[NKV]


def kernel(query, key_, value, mask, q_w, q_b, k_w, k_b, v_w, v_b, o_w, o_b):
    query = np.asarray(query, np.float32)
    key_ = np.asarray(key_, np.float32)
    value = np.asarray(value, np.float32)
    mask = np.asarray(mask)
    q_w = np.asarray(q_w, np.float32)
    q_b = np.asarray(q_b, np.float32)
    k_w = np.asarray(k_w, np.float32)
    k_b = np.asarray(k_b, np.float32)
    v_w = np.asarray(v_w, np.float32)
    v_b = np.asarray(v_b, np.float32)
    o_w = np.asarray(o_w, np.float32)
    o_b = np.asarray(o_b, np.float32)

    counts = (mask != 0).sum(axis=1)
    NKV = max(P, int(-(-int(counts.max()) // P) * P))
    nc = _get_nc(NKV)

    zeros_ob = np.zeros_like(o_b)
    in_maps = []
    for b in range(BS):
        idx = np.nonzero(mask[b])[0]
        cnt = len(idx)
        xk_g = np.zeros((NKV, DIM), np.float32)
        xv_g = np.zeros((NKV, DIM), np.float32)
        xk_g[:cnt] = key_[b][idx]
        xv_g[:cnt] = value[b][idx]
        xqT_b = np.ascontiguousarray(query[b].T).astype(MM_NP)
        xkT_b = np.ascontiguousarray(xk_g.T).astype(MM_NP)
        xvT_b = np.zeros((P * 7, NKV), MM_NP)
        xvT_b[:DIM] = xv_g.T
        xvT_b[DIM] = 1.0
        pb_b = np.where(np.arange(NKV) < cnt, 0.0, NEG).astype(np.float32)
        for g in range(2):
            sl = slice(DGRP * g, DGRP * (g + 1))
            in_maps.append({
                "xqT": xqT_b,
                "xkT": xkT_b,
                "xvT": xvT_b,
                "wqT": np.ascontiguousarray(q_w[sl].T).astype(MM_NP),
                "wkT": np.ascontiguousarray(k_w[sl].T).astype(MM_NP),
                "wvT": np.concatenate(
                    [v_w[sl].T, v_b[None, sl],
                     np.zeros((P - 1, DGRP), np.float32)], axis=0).astype(MM_NP),
                "woT": np.ascontiguousarray(o_w[:, sl].T).astype(MM_NP),
                "qb": q_b[sl].copy(),
                "kb": k_b[sl].copy(),
                "ob": o_b if g == 0 else zeros_ob,
                "pb": pb_b,
            })

    res = run_bass_kernel_spmd(nc, in_maps, core_ids=list(range(N_CORES)),
                               **_run_opts)
    global LAST_RES
    LAST_RES = res
    out = np.empty((BS, SEQ, DIM), np.float32)
    for b in range(BS):
        out[b] = res.results[2 * b]["out"] + res.results[2 * b + 1]["out"]
    return out
